# revision 31
# baseline (speedup 1.0000x reference)
"""Trainium2 Bass kernel for nn_Adapter_CrossNonParam (adapter + prompt/token cross-attention).

Data-parallel over batch: 8 NeuronCores x 4 batches each, adapter weights
replicated, all matmuls bf16 (fp32 PSUM). x is pre-transposed/cast on the host.

v3: explicit per-batch software pipeline so DMA loads (batch b+1) and stores
(batch b) overlap for the whole run -- the DMA roofline for this kernel is
~37MB @ ~350GB/s ~= 105us/core, so every engine must stay below that span.

Emission order per iteration b:
    load x(b+2)                  [sync SWDGE ring]
    down(b+1): 5x8 matmuls into a single recycled PSUM bank, gelu per
               512-chunk (ACT, gelu table)
    attn(b):   per token-tile PAIR j (8): 2 logits MMs into one PSUM bank ->
               one [128,2,200] exp (ACT) -> PE transposes into a dedicated
               bf16 bank -> one pair toktr copy (DVE) -> per tile: 2 up MMs
               + one [128,1024] PSUM->SBUF cast (DVE/ACT mix) -> po pair MMs;
               exp partial sums: accA chain on DVE, accB chain on GpSimd
    tail(b):   partition-major denominator via 4 tiny matmuls, reciprocal,
               prompt up-proj, normalization folded into the epilogue scale
               (2 ACT muls + 2 DVE muls); runs under down(b+2).

ACT table discipline: per-instruction scheduler deps pin the ACT order to
G0 G1 E0 G2 E1 G3 E2 E3 (gelu/exp alternate once per batch -> ~8 table
loads instead of the 25-39 a free-running schedule produced; each load is
~1.3us of ACT time). ACT casts/muls are Copy-class = in every table set.

PSUM ledger (8 banks): down 1, logits(pair) 1, transpose(pair) 1,
up 4 (2 tiles x 2 banks), po+den 1.
"""
import numpy as np
import ml_dtypes

import concourse.bass as bass
import concourse.tile as tile
from concourse import bacc, mybir
from concourse.bass_utils import run_bass_kernel_spmd
from concourse.tile_rust import add_dep_helper

BF = mybir.dt.bfloat16
F32 = mybir.dt.float32

B, N, C = 32, 2248, 1024
D = 128
P = 200
T = N - P  # 2048
NCORES = 8
NB = B // NCORES  # 4 batches per core
SCALE = float(D) ** -0.5

CTILES = C // 128  # 8
TTILES = T // 128  # 16
DOWN_CHUNKS = [(s, min(512, N - s)) for s in range(0, N, 512)]  # 4x512 + 200

# epilogue cast engine per token tile: 5 ACT / 11 DVE (ACT also carries
# gelu+exp+tables; DVE the toktr copies and accA adds)
CAST_ON_ACT = {1, 4, 7, 10, 13}


def build_nc():
    nc = bacc.Bacc("TRN2", target_bir_lowering=False, debug=False, num_devices=NCORES)

    xT = nc.dram_tensor("xT", [NB, C, N], BF, kind="ExternalInput")
    wdn = nc.dram_tensor("wdn", [128, CTILES, 128], BF, kind="ExternalInput")
    wup = nc.dram_tensor("wup", [D, C], BF, kind="ExternalInput")
    bdn = nc.dram_tensor("bdn", [D, 1], F32, kind="ExternalInput")
    ident = nc.dram_tensor("ident", [128, 128], BF, kind="ExternalInput")
    onesf = nc.dram_tensor("onesf", [128, 1], F32, kind="ExternalInput")
    out = nc.dram_tensor("out", [NB, N, C], BF, kind="ExternalOutput")

    with tile.TileContext(nc) as tc:
        with (
            tc.tile_pool(name="const", bufs=1) as const,
            tc.tile_pool(name="xp", bufs=2) as xp,
            tc.tile_pool(name="dg", bufs=3) as dg,
            tc.tile_pool(name="ex", bufs=2) as ex,
            tc.tile_pool(name="tt", bufs=2) as tt,
            tc.tile_pool(name="red", bufs=1) as red,
            tc.tile_pool(name="sm", bufs=1) as sm,
            tc.tile_pool(name="ob", bufs=5) as ob,
            # 8 banks: dn 2 + lg 2 + tr 1 + up 2 + po 1.  dn/lg double-buffered
            # so the ACT chain (gelu after 8 down-MMs, exp after logits pair)
            # stops ping-ponging with the PE on a single bank -- that chain
            # was measured as the span-setting critical path.
            tc.tile_pool(name="ps_dn", bufs=2, space="PSUM") as ps_dn,
            tc.tile_pool(name="ps_lg", bufs=2, space="PSUM") as ps_lg,
            tc.tile_pool(name="ps_tr", bufs=1, space="PSUM") as ps_tr,
            tc.tile_pool(name="ps_up", bufs=2, space="PSUM") as ps_up,
            tc.tile_pool(name="ps_po", bufs=1, space="PSUM") as ps_po,
        ):
            # ---- constants on the sync ring, ahead of the x stream ----
            # (the ACT engine issues its first DMA only at ~7us after boot,
            # which stalled the first down matmul until ~11us; the sync
            # engine issues immediately)
            wdn_sb = const.tile([128, CTILES, 128], BF)
            nc.sync.dma_start(wdn_sb[:], wdn[:])
            bdn_sb = const.tile([D, 1], F32)
            nc.sync.dma_start(bdn_sb[:], bdn[:])
            # the rest is not needed until attention (~15us in); the scalar
            # ring issues them around ~7-10us which is early enough
            id_sb = const.tile([128, 128], BF)
            nc.scalar.dma_start(id_sb[:], ident[:])
            wup_sb = const.tile([D, C], BF)
            nc.scalar.dma_start(wup_sb[:], wup[:])
            onesf_sb = const.tile([128, 1], F32)
            nc.scalar.dma_start(onesf_sb[:], onesf[:])

            xsb_tiles = {}

            def load_x(b, split=False):
                """split=True: n-piece loads so down(0) starts on the first
                512 columns while the rest streams in."""
                xsb = xp.tile([128, CTILES, N], BF, tag="xsb")
                xsb_tiles[b] = xsb
                if split:
                    for s, w in (
                        (0, 128), (128, 128), (256, 256),
                        (512, 512), (1024, 512), (1536, 712),
                    ):
                        src_ap = xT[b, :, s : s + w].rearrange(
                            "(a p) n -> p a n", p=128
                        )
                        nc.sync.dma_start(xsb[:, :, s : s + w], src_ap)
                else:
                    for h in range(2):
                        src = xT[b, h * 512 : (h + 1) * 512, :].rearrange(
                            "(a p) n -> p a n", p=128
                        )
                        nc.sync.dma_start(xsb[:, h * 4 : (h + 1) * 4, :], src)

            def down(b, chunks=DOWN_CHUNKS):
                """Down-projection + gelu. Single PSUM bank recycled per
                512-chunk; the scheduler fills the gelu-wait gaps with the
                concurrent attn(b-1) matmuls."""
                xsb = xsb_tiles[b]
                dng = dg.tile([128, N], BF, tag="dng")
                gelus = []
                for s, w in chunks:
                    acc_full = ps_dn.tile([128, 512], F32, tag="dn")
                    acc = acc_full[:, :w]
                    for c in range(CTILES):
                        nc.tensor.matmul(
                            acc[:],
                            wdn_sb[:, c, :],
                            xsb[:, c, s : s + w],
                            start=(c == 0),
                            stop=(c == CTILES - 1),
                        )
                    g = nc.scalar.activation(
                        dng[:, s : s + w],
                        acc[:],
                        mybir.ActivationFunctionType.Gelu,
                        bias=bdn_sb[:],
                        scale=1.0,
                    )
                    gelus.append(g)
                return dng, gelus

            def attn_scores(b, dng):
                """Logits, exp, transposes, exp partial sums. Emitted BEFORE
                down(b+1) so the exps fill the ACT while down(b+1) waits on
                its x load; the PE cost here is small (~3us)."""
                exps = ex.tile([128, TTILES, P], BF, tag="exps", name="exps")
                toktr = tt.tile([128, TTILES, 128], BF, tag="toktr", name="toktr")
                poT = ps_po.tile([128, P + 8], F32, tag="po", name="poT")
                accA = red.tile([128, P], F32, tag="accA", name="accA")
                accB = red.tile([128, P], F32, tag="accB", name="accB")
                exp_insts = []
                st = {
                    "exps": exps, "toktr": toktr, "poT": poT,
                    "accA": accA, "accB": accB, "exp_insts": exp_insts,
                }
                for j in range(TTILES // 2):
                    t0, t1 = 2 * j, 2 * j + 1
                    lg = ps_lg.tile([128, 2, P], F32, tag="lg")
                    for k, t in ((0, t0), (1, t1)):
                        tok = dng[:, P + t * 128 : P + (t + 1) * 128]
                        nc.tensor.matmul(
                            lg[:, k, :], tok, dng[:, 0:P], start=True, stop=True
                        )
                    e = nc.scalar.activation(
                        exps[:, t0 : t0 + 2, :],
                        lg[:],
                        mybir.ActivationFunctionType.Exp,
                        scale=SCALE,
                    )
                    exp_insts.append(e)
                    # pair of PE transposes into the dedicated bf16 bank,
                    # one DVE copy for both
                    trp = ps_tr.tile([128, 2, 128], BF, tag="tr")
                    for k, t in ((0, t0), (1, t1)):
                        tok = dng[:, P + t * 128 : P + (t + 1) * 128]
                        nc.tensor.transpose(trp[:, k, :], tok, id_sb[:])
                    nc.vector.tensor_copy(toktr[:, t0 : t0 + 2, :], trp[:])
                    # exp partial sums: accA chain on DVE, accB on GpSimd
                    if j == 0:
                        nc.vector.tensor_add(accA[:], exps[:, 0, :], exps[:, 1, :])
                    elif j == 1:
                        nc.gpsimd.tensor_add(accB[:], exps[:, 2, :], exps[:, 3, :])
                    else:
                        nc.vector.tensor_add(accA[:], accA[:], exps[:, 2 * j, :])
                        nc.gpsimd.tensor_add(accB[:], accB[:], exps[:, 2 * j + 1, :])
                return st

            def tail(b, dng, poT, accA, accB):
                """Attention tail: denominator, reciprocal, prompt up-proj
                with normalization folded into the epilogue scale."""
                # accA += accB on GpSimd (SBUF-only) halves the den matmuls
                nc.gpsimd.tensor_add(accA[:], accA[:], accB[:])
                nc.tensor.matmul(
                    poT[:, P : P + 1], accA[:, 0:128], onesf_sb[:],
                    start=True, stop=True,
                )
                nc.tensor.matmul(
                    poT[0:72, P + 1 : P + 2], accA[:, 128:200], onesf_sb[:],
                    start=True, stop=True,
                )
                rec0 = sm.tile([128, 1], F32, tag="rec0")
                nc.vector.reciprocal(rec0[:], poT[:, P : P + 1])
                rec1 = sm.tile([72, 1], F32, tag="rec1")
                nc.vector.reciprocal(rec1[:], poT[0:72, P + 1 : P + 2])
                # unnormalized prompt_out -> dng's prompt region (DVE)
                nc.vector.tensor_copy(dng[:, 0:P], poT[:, 0:P])

                osbp = ob.tile([128, 4, 2, 512], BF, tag="osb")
                for h in range(2):
                    upa = ps_up.tile([128, 512], F32, tag="up")
                    nc.tensor.matmul(
                        upa[:],
                        dng[:, 0:128],
                        wup_sb[:, h * 512 : (h + 1) * 512],
                        start=True, stop=True,
                    )
                    nc.scalar.mul(osbp[:, 0, h, :], upa[:], rec0[:])
                    upb = ps_up.tile([128, 512], F32, tag="up")
                    nc.tensor.matmul(
                        upb[0:72, :],
                        dng[:, 128:200],
                        wup_sb[:, h * 512 : (h + 1) * 512],
                        start=True, stop=True,
                    )
                    nc.vector.tensor_scalar_mul(
                        osbp[0:72, 1, h, :], upb[0:72, :], rec1[:]
                    )
                nc.gpsimd.dma_start(
                    out[b, 0:128, :], osbp[:, 0].rearrange("p a b -> p (a b)")
                )
                nc.gpsimd.dma_start(
                    out[b, 128:200, :], osbp[0:72, 1].rearrange("p a b -> p (a b)")
                )

            def attn_finish(b, dng, st, late_up=False):
                """po accumulation block, tail, then the token up-projection
                stream. late_up (last batch): borrow the idle down/logits/
                transpose banks and store at 2-tile granularity."""
                exps, toktr, poT = st["exps"], st["toktr"], st["poT"]
                for t in range(TTILES):
                    nc.tensor.matmul(
                        poT[:, 0:P],
                        toktr[:, t, :],
                        exps[:, t, :],
                        start=(t == 0),
                        stop=(t == TTILES - 1),
                    )
                tail(b, dng, poT, st["accA"], st["accB"])

                borrow = [
                    (ps_up, "up"), (ps_up, "up"), (ps_dn, "dn"), (ps_dn, "dn"),
                    (ps_lg, "lg"), (ps_lg, "lg"), (ps_tr, "tr"),
                ]
                bctr = [0]
                osb = None
                for t in range(TTILES):
                    tok = dng[:, P + t * 128 : P + (t + 1) * 128]
                    q = t % 4
                    if q == 0:
                        osb = ob.tile([128, 4, 2, 512], BF, tag="osb")
                    # single-bank up tiles; the PSUM->SBUF casts run on ACT
                    # and DVE in parallel so each bank frees fast
                    ha = t % 2
                    for h in range(2):
                        if late_up:
                            pool, tg = borrow[bctr[0] % len(borrow)]
                            bctr[0] += 1
                            up = pool.tile([128, 512], F32, tag=tg)
                        else:
                            up = ps_up.tile([128, 512], F32, tag="up")
                        nc.tensor.matmul(
                            up[:],
                            tok,
                            wup_sb[:, h * 512 : (h + 1) * 512],
                            start=True,
                            stop=True,
                        )
                        if h == ha:
                            nc.scalar.copy(osb[:, q, h, :], up[:])
                        else:
                            nc.vector.tensor_copy(osb[:, q, h, :], up[:])
                    if late_up and q in (1, 3):
                        g2 = t // 2
                        dstd = out[
                            b, P + 256 * g2 : P + 256 * (g2 + 1), :
                        ].rearrange("(a p) c -> p a c", p=128)
                        lo = 0 if q == 1 else 2
                        nc.gpsimd.dma_start(
                            dstd,
                            osb[:, lo : lo + 2].rearrange("p a b c -> p a (b c)"),
                        )
                    elif not late_up and q == 3:
                        g4 = t // 4
                        dstd = out[
                            b, P + 512 * g4 : P + 512 * (g4 + 1), :
                        ].rearrange("(a p) c -> p a c", p=128)
                        nc.gpsimd.dma_start(
                            dstd, osb[:].rearrange("p a b c -> p a (b c)")
                        )

            # ---- software pipeline ----
            # per iteration b: [load x(b+2)] [attn_scores(b): logits+exps]
            # [down(b+1)] [attn_finish(b): po, tail, up+stores]
            # ACT order G0 E0 G1 E1 ... : each E(b) needs only logits(b), so
            # it runs while down(b+1) waits on its x load.
            load_x(0, split=True)
            load_x(1)
            dngs = {}
            gelu_groups = {}
            exp_groups = {}
            # batch 0's leading chunks are finer so the PE starts as soon as
            # the first 128 columns of x(0) land (~2us instead of ~7us)
            CHUNKS0 = [(0, 128), (128, 128), (256, 256)] + DOWN_CHUNKS[1:]
            dngs[0], gelu_groups[0] = down(0, chunks=CHUNKS0)
            for b in range(NB):
                if b + 2 < NB:
                    load_x(b + 2)
                st = attn_scores(b, dngs[b])
                exp_groups[b] = st["exp_insts"]
                if b + 1 < NB:
                    dngs[b + 1], gelu_groups[b + 1] = down(b + 1)
                attn_finish(b, dngs[b], st, late_up=(b == NB - 1))

            # ACT table discipline: pin the per-engine order G0 E0 G1 E1 ...
            # (gelu/exp alternate once per batch; per-instruction edges so
            # the scheduler cannot interleave the groups)
            for b in range(NB):
                g_last = gelu_groups[b][-1].ins
                for e in exp_groups[b]:
                    add_dep_helper(
                        e.ins, g_last, sync=False,
                        reason="ACT order: exps(b) after gelus(b)",
                    )
                if b + 1 < NB:
                    e_last = exp_groups[b][-1].ins
                    for g in gelu_groups[b + 1]:
                        add_dep_helper(
                            g.ins, e_last, sync=False,
                            reason="ACT order: gelus(b+1) after exps(b)",
                        )

    nc.compile()
    return nc


_NC_CACHE = None


def _get_nc():
    global _NC_CACHE
    if _NC_CACHE is None:
        _NC_CACHE = build_nc()
    return _NC_CACHE


def make_in_maps(x, W_down, b_down, W_up, b_up, gate):
    x = np.asarray(x, np.float32)
    W_down = np.asarray(W_down, np.float32)
    b_down = np.asarray(b_down, np.float32)
    W_up = np.asarray(W_up, np.float32)
    b_up = np.asarray(b_up, np.float32)
    gate = float(np.asarray(gate, np.float32))

    bf = ml_dtypes.bfloat16
    xT = np.ascontiguousarray(x.transpose(0, 2, 1)).astype(bf)  # [B, C, N]
    # wdn[p, c, m] = W_down[c*128 + p, m]
    wdn = np.ascontiguousarray(
        W_down.reshape(CTILES, 128, 128).transpose(1, 0, 2)
    ).astype(bf)
    wup = (W_up * gate).astype(bf)  # [D, C]
    bdn = b_down.reshape(D, 1).copy()
    ident = np.eye(128, dtype=bf)
    onesf = np.ones((128, 1), dtype=np.float32)

    in_maps = []
    for i in range(NCORES):
        in_maps.append(
            {
                "xT": np.ascontiguousarray(xT[i * NB : (i + 1) * NB]),
                "wdn": wdn,
                "wup": wup,
                "bdn": bdn,
                "ident": ident,
                "onesf": onesf,
            }
        )
    return in_maps


def kernel(**inputs):
    nc = _get_nc()
    in_maps = make_in_maps(**inputs)
    res = run_bass_kernel_spmd(nc, in_maps, core_ids=list(range(NCORES)))
    out = np.concatenate([res.results[i]["out"] for i in range(NCORES)], axis=0)
    out = out.astype(np.float32)
    # b_up (and gate) folded in on the host: device computes comb @ (gate*W_up)
    bias = (
        np.asarray(inputs["b_up"], np.float32)
        * float(np.asarray(inputs["gate"], np.float32))
    ).reshape(1, 1, C)
    return out + bias


# revision 33
# speedup vs baseline: 1.0444x; 1.0444x over previous
"""Trainium2 Bass kernel for nn_Adapter_CrossNonParam (adapter + prompt/token cross-attention).

Data-parallel over batch: 8 NeuronCores x 4 batches each, adapter weights
replicated, all matmuls bf16 (fp32 PSUM). x is pre-transposed/cast on the host.

v3: explicit per-batch software pipeline so DMA loads (batch b+1) and stores
(batch b) overlap for the whole run -- the DMA roofline for this kernel is
~37MB @ ~350GB/s ~= 105us/core, so every engine must stay below that span.

Emission order per iteration b:
    load x(b+2)                  [sync SWDGE ring]
    down(b+1): 5x8 matmuls into a single recycled PSUM bank, gelu per
               512-chunk (ACT, gelu table)
    attn(b):   per token-tile PAIR j (8): 2 logits MMs into one PSUM bank ->
               one [128,2,200] exp (ACT) -> PE transposes into a dedicated
               bf16 bank -> one pair toktr copy (DVE) -> per tile: 2 up MMs
               + one [128,1024] PSUM->SBUF cast (DVE/ACT mix) -> po pair MMs;
               exp partial sums: accA chain on DVE, accB chain on GpSimd
    tail(b):   partition-major denominator via 4 tiny matmuls, reciprocal,
               prompt up-proj, normalization folded into the epilogue scale
               (2 ACT muls + 2 DVE muls); runs under down(b+2).

ACT table discipline: per-instruction scheduler deps pin the ACT order to
G0 G1 E0 G2 E1 G3 E2 E3 (gelu/exp alternate once per batch -> ~8 table
loads instead of the 25-39 a free-running schedule produced; each load is
~1.3us of ACT time). ACT casts/muls are Copy-class = in every table set.

PSUM ledger (8 banks): down 1, logits(pair) 1, transpose(pair) 1,
up 4 (2 tiles x 2 banks), po+den 1.
"""
import numpy as np
import ml_dtypes

import concourse.bass as bass
import concourse.tile as tile
from concourse import bacc, mybir
from concourse.bass_utils import run_bass_kernel_spmd
from concourse.tile_rust import add_dep_helper

BF = mybir.dt.bfloat16
F32 = mybir.dt.float32

B, N, C = 32, 2248, 1024
D = 128
P = 200
T = N - P  # 2048
NCORES = 8
NB = B // NCORES  # 4 batches per core
SCALE = float(D) ** -0.5

CTILES = C // 128  # 8
TTILES = T // 128  # 16
DOWN_CHUNKS = [(s, min(512, N - s)) for s in range(0, N, 512)]  # 4x512 + 200

# epilogue cast engine per token tile: 5 ACT / 11 DVE (ACT also carries
# gelu+exp+tables; DVE the toktr copies and accA adds)
CAST_ON_ACT = {1, 4, 7, 10, 13}


def build_nc():
    nc = bacc.Bacc("TRN2", target_bir_lowering=False, debug=False, num_devices=NCORES)

    xT = nc.dram_tensor("xT", [NB, C, N], BF, kind="ExternalInput")
    wdn = nc.dram_tensor("wdn", [128, CTILES, 128], BF, kind="ExternalInput")
    wup = nc.dram_tensor("wup", [D, C], BF, kind="ExternalInput")
    bdn = nc.dram_tensor("bdn", [D, 1], F32, kind="ExternalInput")
    ident = nc.dram_tensor("ident", [128, 128], BF, kind="ExternalInput")
    onesf = nc.dram_tensor("onesf", [128, 1], F32, kind="ExternalInput")
    out = nc.dram_tensor("out", [NB, N, C], BF, kind="ExternalOutput")

    with tile.TileContext(nc) as tc:
        with (
            tc.tile_pool(name="const", bufs=1) as const,
            tc.tile_pool(name="xp", bufs=2) as xp,
            tc.tile_pool(name="dg", bufs=3) as dg,
            tc.tile_pool(name="ex", bufs=2) as ex,
            tc.tile_pool(name="tt", bufs=2) as tt,
            tc.tile_pool(name="red", bufs=1) as red,
            tc.tile_pool(name="sm", bufs=1) as sm,
            tc.tile_pool(name="ob", bufs=5) as ob,
            # 8 banks: dn 2 + lg 2 + tr 1 + up 2 + po 1.  dn/lg double-buffered
            # so the ACT chain (gelu after 8 down-MMs, exp after logits pair)
            # stops ping-ponging with the PE on a single bank -- that chain
            # was measured as the span-setting critical path.
            tc.tile_pool(name="ps_dn", bufs=2, space="PSUM") as ps_dn,
            tc.tile_pool(name="ps_lg", bufs=2, space="PSUM") as ps_lg,
            tc.tile_pool(name="ps_tr", bufs=1, space="PSUM") as ps_tr,
            tc.tile_pool(name="ps_up", bufs=2, space="PSUM") as ps_up,
            tc.tile_pool(name="ps_po", bufs=1, space="PSUM") as ps_po,
        ):
            # ---- constants on the sync ring, ahead of the x stream ----
            # (the ACT engine issues its first DMA only at ~7us after boot,
            # which stalled the first down matmul until ~11us; the sync
            # engine issues immediately)
            wdn_sb = const.tile([128, CTILES, 128], BF)
            nc.sync.dma_start(wdn_sb[:], wdn[:])
            bdn_sb = const.tile([D, 1], F32)
            nc.sync.dma_start(bdn_sb[:], bdn[:])
            # the rest is not needed until attention (~15us in); the scalar
            # ring issues them around ~7-10us which is early enough
            id_sb = const.tile([128, 128], BF)
            nc.scalar.dma_start(id_sb[:], ident[:])
            wup_sb = const.tile([D, C], BF)
            nc.scalar.dma_start(wup_sb[:], wup[:])
            onesf_sb = const.tile([128, 1], F32)
            nc.scalar.dma_start(onesf_sb[:], onesf[:])

            xsb_tiles = {}

            def load_x(b, split=False):
                """split=True: n-piece loads so down(0) starts on the first
                512 columns while the rest streams in."""
                xsb = xp.tile([128, CTILES, N], BF, tag="xsb")
                xsb_tiles[b] = xsb
                if split:
                    for s, w in (
                        (0, 128), (128, 128), (256, 256),
                        (512, 512), (1024, 512), (1536, 712),
                    ):
                        src_ap = xT[b, :, s : s + w].rearrange(
                            "(a p) n -> p a n", p=128
                        )
                        nc.sync.dma_start(xsb[:, :, s : s + w], src_ap)
                else:
                    for h in range(2):
                        src = xT[b, h * 512 : (h + 1) * 512, :].rearrange(
                            "(a p) n -> p a n", p=128
                        )
                        nc.sync.dma_start(xsb[:, h * 4 : (h + 1) * 4, :], src)

            def down(b, chunks=DOWN_CHUNKS):
                """Down-projection + gelu. Single PSUM bank recycled per
                512-chunk; the scheduler fills the gelu-wait gaps with the
                concurrent attn(b-1) matmuls."""
                xsb = xsb_tiles[b]
                dng = dg.tile([128, N], BF, tag="dng")
                gelus = []
                for s, w in chunks:
                    acc_full = ps_dn.tile([128, 512], F32, tag="dn")
                    acc = acc_full[:, :w]
                    for c in range(CTILES):
                        nc.tensor.matmul(
                            acc[:],
                            wdn_sb[:, c, :],
                            xsb[:, c, s : s + w],
                            start=(c == 0),
                            stop=(c == CTILES - 1),
                        )
                    g = nc.scalar.activation(
                        dng[:, s : s + w],
                        acc[:],
                        mybir.ActivationFunctionType.Gelu,
                        bias=bdn_sb[:],
                        scale=1.0,
                    )
                    gelus.append(g)
                return dng, gelus

            def attn_scores(b, dng):
                """Logits, exp, transposes, exp partial sums. Emitted BEFORE
                down(b+1) so the exps fill the ACT while down(b+1) waits on
                its x load; the PE cost here is small (~3us)."""
                exps = ex.tile([128, TTILES, P], BF, tag="exps", name="exps")
                toktr = tt.tile([128, TTILES, 128], BF, tag="toktr", name="toktr")
                poT = ps_po.tile([128, P + 8], F32, tag="po", name="poT")
                accA = red.tile([128, P], F32, tag="accA", name="accA")
                accB = red.tile([128, P], F32, tag="accB", name="accB")
                exp_insts = []
                st = {
                    "exps": exps, "toktr": toktr, "poT": poT,
                    "accA": accA, "accB": accB, "exp_insts": exp_insts,
                }
                for j in range(TTILES // 2):
                    t0, t1 = 2 * j, 2 * j + 1
                    lg = ps_lg.tile([128, 2, P], F32, tag="lg")
                    for k, t in ((0, t0), (1, t1)):
                        tok = dng[:, P + t * 128 : P + (t + 1) * 128]
                        nc.tensor.matmul(
                            lg[:, k, :], tok, dng[:, 0:P], start=True, stop=True
                        )
                    e = nc.scalar.activation(
                        exps[:, t0 : t0 + 2, :],
                        lg[:],
                        mybir.ActivationFunctionType.Exp,
                        scale=SCALE,
                    )
                    exp_insts.append(e)
                    # pair of PE transposes into the dedicated bf16 bank,
                    # one DVE copy for both
                    trp = ps_tr.tile([128, 2, 128], BF, tag="tr")
                    for k, t in ((0, t0), (1, t1)):
                        tok = dng[:, P + t * 128 : P + (t + 1) * 128]
                        nc.tensor.transpose(trp[:, k, :], tok, id_sb[:])
                    nc.vector.tensor_copy(toktr[:, t0 : t0 + 2, :], trp[:])
                    # exp partial sums: accA chain on DVE, accB on GpSimd
                    if j == 0:
                        nc.vector.tensor_add(accA[:], exps[:, 0, :], exps[:, 1, :])
                    elif j == 1:
                        nc.gpsimd.tensor_add(accB[:], exps[:, 2, :], exps[:, 3, :])
                    else:
                        nc.vector.tensor_add(accA[:], accA[:], exps[:, 2 * j, :])
                        nc.gpsimd.tensor_add(accB[:], accB[:], exps[:, 2 * j + 1, :])
                return st

            def tail(b, dng, poT, accA, accB):
                """Attention tail: denominator, reciprocal, prompt up-proj
                with normalization folded into the epilogue scale."""
                # accA += accB on GpSimd (SBUF-only) halves the den matmuls
                nc.gpsimd.tensor_add(accA[:], accA[:], accB[:])
                nc.tensor.matmul(
                    poT[:, P : P + 1], accA[:, 0:128], onesf_sb[:],
                    start=True, stop=True,
                )
                nc.tensor.matmul(
                    poT[0:72, P + 1 : P + 2], accA[:, 128:200], onesf_sb[:],
                    start=True, stop=True,
                )
                rec0 = sm.tile([128, 1], F32, tag="rec0")
                nc.vector.reciprocal(rec0[:], poT[:, P : P + 1])
                rec1 = sm.tile([72, 1], F32, tag="rec1")
                nc.vector.reciprocal(rec1[:], poT[0:72, P + 1 : P + 2])
                # unnormalized prompt_out -> dng's prompt region (DVE)
                nc.vector.tensor_copy(dng[:, 0:P], poT[:, 0:P])

                osbp = ob.tile([128, 4, 2, 512], BF, tag="osb")
                for h in range(2):
                    upa = ps_up.tile([128, 512], F32, tag="up")
                    nc.tensor.matmul(
                        upa[:],
                        dng[:, 0:128],
                        wup_sb[:, h * 512 : (h + 1) * 512],
                        start=True, stop=True,
                    )
                    nc.scalar.mul(osbp[:, 0, h, :], upa[:], rec0[:])
                    upb = ps_up.tile([128, 512], F32, tag="up")
                    nc.tensor.matmul(
                        upb[0:72, :],
                        dng[:, 128:200],
                        wup_sb[:, h * 512 : (h + 1) * 512],
                        start=True, stop=True,
                    )
                    nc.vector.tensor_scalar_mul(
                        osbp[0:72, 1, h, :], upb[0:72, :], rec1[:]
                    )
                nc.gpsimd.dma_start(
                    out[b, 0:128, :], osbp[:, 0].rearrange("p a b -> p (a b)")
                )
                nc.gpsimd.dma_start(
                    out[b, 128:200, :], osbp[0:72, 1].rearrange("p a b -> p (a b)")
                )

            def attn_finish(b, dng, st, late_up=False):
                """po accumulation block, tail, then the token up-projection
                stream. late_up (last batch): borrow the idle down/logits/
                transpose banks and store at 2-tile granularity."""
                exps, toktr, poT = st["exps"], st["toktr"], st["poT"]

                def po_and_tail():
                    for t in range(TTILES):
                        nc.tensor.matmul(
                            poT[:, 0:P],
                            toktr[:, t, :],
                            exps[:, t, :],
                            start=(t == 0),
                            stop=(t == TTILES - 1),
                        )
                    tail(b, dng, poT, st["accA"], st["accB"])

                if late_up:
                    # drain mode: the long po->den->prompt-up chain first so
                    # it overlaps the final up/cast/store stream
                    po_and_tail()

                borrow = [
                    (ps_up, "up"), (ps_up, "up"), (ps_dn, "dn"), (ps_dn, "dn"),
                    (ps_lg, "lg"), (ps_lg, "lg"), (ps_tr, "tr"),
                ]
                bctr = [0]
                osb = None
                for t in range(TTILES):
                    tok = dng[:, P + t * 128 : P + (t + 1) * 128]
                    q = t % 4
                    if q == 0:
                        osb = ob.tile([128, 4, 2, 512], BF, tag="osb")
                    # single-bank up tiles; the PSUM->SBUF casts run on ACT
                    # and DVE in parallel so each bank frees fast
                    ha = t % 2
                    for h in range(2):
                        if late_up:
                            pool, tg = borrow[bctr[0] % len(borrow)]
                            bctr[0] += 1
                            up = pool.tile([128, 512], F32, tag=tg)
                        else:
                            up = ps_up.tile([128, 512], F32, tag="up")
                        nc.tensor.matmul(
                            up[:],
                            tok,
                            wup_sb[:, h * 512 : (h + 1) * 512],
                            start=True,
                            stop=True,
                        )
                        if h == ha:
                            nc.scalar.copy(osb[:, q, h, :], up[:])
                        else:
                            nc.vector.tensor_copy(osb[:, q, h, :], up[:])
                    if late_up and q in (1, 3):
                        g2 = t // 2
                        dstd = out[
                            b, P + 256 * g2 : P + 256 * (g2 + 1), :
                        ].rearrange("(a p) c -> p a c", p=128)
                        lo = 0 if q == 1 else 2
                        nc.gpsimd.dma_start(
                            dstd,
                            osb[:, lo : lo + 2].rearrange("p a b c -> p a (b c)"),
                        )
                    elif not late_up and q == 3:
                        g4 = t // 4
                        dstd = out[
                            b, P + 512 * g4 : P + 512 * (g4 + 1), :
                        ].rearrange("(a p) c -> p a c", p=128)
                        nc.gpsimd.dma_start(
                            dstd, osb[:].rearrange("p a b c -> p a (b c)")
                        )
                if not late_up:
                    po_and_tail()

            # ---- software pipeline ----
            # per iteration b: [load x(b+2)] [attn_scores(b): logits+exps]
            # [down(b+1)] [attn_finish(b): po, tail, up+stores]
            # ACT order G0 E0 G1 E1 ... : each E(b) needs only logits(b), so
            # it runs while down(b+1) waits on its x load.
            load_x(0, split=True)
            load_x(1)
            dngs = {}
            gelu_groups = {}
            exp_groups = {}
            # batch 0's leading chunks are finer so the PE starts as soon as
            # the first 128 columns of x(0) land (~2us instead of ~7us)
            CHUNKS0 = [(0, 128), (128, 128), (256, 256)] + DOWN_CHUNKS[1:]
            dngs[0], gelu_groups[0] = down(0, chunks=CHUNKS0)
            for b in range(NB):
                if b + 2 < NB:
                    load_x(b + 2)
                st = attn_scores(b, dngs[b])
                exp_groups[b] = st["exp_insts"]
                if b + 1 < NB:
                    dngs[b + 1], gelu_groups[b + 1] = down(b + 1)
                attn_finish(b, dngs[b], st, late_up=(b == NB - 1))

            # ACT table discipline: pin the per-engine order G0 E0 G1 E1 ...
            # (gelu/exp alternate once per batch; per-instruction edges so
            # the scheduler cannot interleave the groups)
            for b in range(NB):
                g_last = gelu_groups[b][-1].ins
                for e in exp_groups[b]:
                    add_dep_helper(
                        e.ins, g_last, sync=False,
                        reason="ACT order: exps(b) after gelus(b)",
                    )
                if b + 1 < NB:
                    e_last = exp_groups[b][-1].ins
                    for g in gelu_groups[b + 1]:
                        add_dep_helper(
                            g.ins, e_last, sync=False,
                            reason="ACT order: gelus(b+1) after exps(b)",
                        )

    nc.compile()
    return nc


_NC_CACHE = None


def _get_nc():
    global _NC_CACHE
    if _NC_CACHE is None:
        _NC_CACHE = build_nc()
    return _NC_CACHE


def make_in_maps(x, W_down, b_down, W_up, b_up, gate):
    x = np.asarray(x, np.float32)
    W_down = np.asarray(W_down, np.float32)
    b_down = np.asarray(b_down, np.float32)
    W_up = np.asarray(W_up, np.float32)
    b_up = np.asarray(b_up, np.float32)
    gate = float(np.asarray(gate, np.float32))

    bf = ml_dtypes.bfloat16
    xT = np.ascontiguousarray(x.transpose(0, 2, 1)).astype(bf)  # [B, C, N]
    # wdn[p, c, m] = W_down[c*128 + p, m]
    wdn = np.ascontiguousarray(
        W_down.reshape(CTILES, 128, 128).transpose(1, 0, 2)
    ).astype(bf)
    wup = (W_up * gate).astype(bf)  # [D, C]
    bdn = b_down.reshape(D, 1).copy()
    ident = np.eye(128, dtype=bf)
    onesf = np.ones((128, 1), dtype=np.float32)

    in_maps = []
    for i in range(NCORES):
        in_maps.append(
            {
                "xT": np.ascontiguousarray(xT[i * NB : (i + 1) * NB]),
                "wdn": wdn,
                "wup": wup,
                "bdn": bdn,
                "ident": ident,
                "onesf": onesf,
            }
        )
    return in_maps


def kernel(**inputs):
    nc = _get_nc()
    in_maps = make_in_maps(**inputs)
    res = run_bass_kernel_spmd(nc, in_maps, core_ids=list(range(NCORES)))
    out = np.concatenate([res.results[i]["out"] for i in range(NCORES)], axis=0)
    out = out.astype(np.float32)
    # b_up (and gate) folded in on the host: device computes comb @ (gate*W_up)
    bias = (
        np.asarray(inputs["b_up"], np.float32)
        * float(np.asarray(inputs["gate"], np.float32))
    ).reshape(1, 1, C)
    return out + bias


# revision 34
# speedup vs baseline: 1.0961x; 1.0495x over previous
"""Trainium2 Bass kernel for nn_Adapter_CrossNonParam (adapter + prompt/token cross-attention).

Data-parallel over batch: 8 NeuronCores x 4 batches each, adapter weights
replicated, all matmuls bf16 (fp32 PSUM). x is pre-transposed/cast on the host.

v3: explicit per-batch software pipeline so DMA loads (batch b+1) and stores
(batch b) overlap for the whole run -- the DMA roofline for this kernel is
~37MB @ ~350GB/s ~= 105us/core, so every engine must stay below that span.

Emission order per iteration b:
    load x(b+2)                  [sync SWDGE ring]
    down(b+1): 5x8 matmuls into a single recycled PSUM bank, gelu per
               512-chunk (ACT, gelu table)
    attn(b):   per token-tile PAIR j (8): 2 logits MMs into one PSUM bank ->
               one [128,2,200] exp (ACT) -> PE transposes into a dedicated
               bf16 bank -> one pair toktr copy (DVE) -> per tile: 2 up MMs
               + one [128,1024] PSUM->SBUF cast (DVE/ACT mix) -> po pair MMs;
               exp partial sums: accA chain on DVE, accB chain on GpSimd
    tail(b):   partition-major denominator via 4 tiny matmuls, reciprocal,
               prompt up-proj, normalization folded into the epilogue scale
               (2 ACT muls + 2 DVE muls); runs under down(b+2).

ACT table discipline: per-instruction scheduler deps pin the ACT order to
G0 G1 E0 G2 E1 G3 E2 E3 (gelu/exp alternate once per batch -> ~8 table
loads instead of the 25-39 a free-running schedule produced; each load is
~1.3us of ACT time). ACT casts/muls are Copy-class = in every table set.

PSUM ledger (8 banks): down 1, logits(pair) 1, transpose(pair) 1,
up 4 (2 tiles x 2 banks), po+den 1.
"""
import numpy as np
import ml_dtypes

import concourse.bass as bass
import concourse.tile as tile
from concourse import bacc, mybir
from concourse.bass_utils import run_bass_kernel_spmd
from concourse.tile_rust import add_dep_helper

BF = mybir.dt.bfloat16
F32 = mybir.dt.float32

B, N, C = 32, 2248, 1024
D = 128
P = 200
T = N - P  # 2048
NCORES = 8
NB = B // NCORES  # 4 batches per core
SCALE = float(D) ** -0.5

CTILES = C // 128  # 8
TTILES = T // 128  # 16
DOWN_CHUNKS = [(s, min(512, N - s)) for s in range(0, N, 512)]  # 4x512 + 200

# epilogue cast engine per token tile: 5 ACT / 11 DVE (ACT also carries
# gelu+exp+tables; DVE the toktr copies and accA adds)
CAST_ON_ACT = {1, 4, 7, 10, 13}


def build_nc():
    nc = bacc.Bacc("TRN2", target_bir_lowering=False, debug=False, num_devices=NCORES)

    xT = nc.dram_tensor("xT", [NB, C, N], BF, kind="ExternalInput")
    wdn = nc.dram_tensor("wdn", [128, CTILES, 128], BF, kind="ExternalInput")
    wup = nc.dram_tensor("wup", [D, C], BF, kind="ExternalInput")
    bdn = nc.dram_tensor("bdn", [D, 1], F32, kind="ExternalInput")
    ident = nc.dram_tensor("ident", [128, 128], BF, kind="ExternalInput")
    onesf = nc.dram_tensor("onesf", [128, 1], F32, kind="ExternalInput")
    out = nc.dram_tensor("out", [NB, N, C], BF, kind="ExternalOutput")

    with tile.TileContext(nc) as tc:
        with (
            tc.tile_pool(name="const", bufs=1) as const,
            tc.tile_pool(name="xp", bufs=2) as xp,
            tc.tile_pool(name="dg", bufs=3) as dg,
            tc.tile_pool(name="ex", bufs=2) as ex,
            tc.tile_pool(name="tt", bufs=2) as tt,
            tc.tile_pool(name="red", bufs=1) as red,
            tc.tile_pool(name="sm", bufs=1) as sm,
            tc.tile_pool(name="ob", bufs=5) as ob,
            # 8 banks: dn 2 + lg 2 + tr 1 + up 2 + po 1.  dn/lg double-buffered
            # so the ACT chain (gelu after 8 down-MMs, exp after logits pair)
            # stops ping-ponging with the PE on a single bank -- that chain
            # was measured as the span-setting critical path.
            tc.tile_pool(name="ps_dn", bufs=2, space="PSUM") as ps_dn,
            tc.tile_pool(name="ps_lg", bufs=2, space="PSUM") as ps_lg,
            tc.tile_pool(name="ps_tr", bufs=1, space="PSUM") as ps_tr,
            tc.tile_pool(name="ps_up", bufs=2, space="PSUM") as ps_up,
            tc.tile_pool(name="ps_po", bufs=1, space="PSUM") as ps_po,
        ):
            # ---- constants on the sync ring, ahead of the x stream ----
            # (the ACT engine issues its first DMA only at ~7us after boot,
            # which stalled the first down matmul until ~11us; the sync
            # engine issues immediately)
            wdn_sb = const.tile([128, CTILES, 128], BF)
            nc.sync.dma_start(wdn_sb[:], wdn[:])
            bdn_sb = const.tile([D, 1], F32)
            nc.sync.dma_start(bdn_sb[:], bdn[:])
            # the rest is not needed until attention (~15us in); the scalar
            # ring issues them around ~7-10us which is early enough
            id_sb = const.tile([128, 128], BF)
            nc.scalar.dma_start(id_sb[:], ident[:])
            wup_sb = const.tile([D, C], BF)
            nc.scalar.dma_start(wup_sb[:], wup[:])
            onesf_sb = const.tile([128, 1], F32)
            nc.scalar.dma_start(onesf_sb[:], onesf[:])

            xsb_tiles = {}

            def load_x(b, split=False):
                """split=True: n-piece loads so down(0) starts on the first
                512 columns while the rest streams in."""
                xsb = xp.tile([128, CTILES, N], BF, tag="xsb")
                xsb_tiles[b] = xsb
                # n-piece splits: each down chunk's 8-ctile accumulation can
                # finish as soon as ITS columns land, so the gelus spread
                # across the load window instead of bunching at load end
                # (a c-split load makes every chunk wait for the full 4.6MB)
                if split:
                    pieces = (
                        (0, 128), (128, 128), (256, 256),
                        (512, 512), (1024, 512), (1536, 712),
                    )
                else:
                    pieces = ((0, 512), (512, 512), (1024, 512), (1536, 712))
                for s, w in pieces:
                    src_ap = xT[b, :, s : s + w].rearrange(
                        "(a p) n -> p a n", p=128
                    )
                    nc.sync.dma_start(xsb[:, :, s : s + w], src_ap)

            def down(b, chunks=DOWN_CHUNKS):
                """Down-projection + gelu. Single PSUM bank recycled per
                512-chunk; the scheduler fills the gelu-wait gaps with the
                concurrent attn(b-1) matmuls."""
                xsb = xsb_tiles[b]
                dng = dg.tile([128, N], BF, tag="dng")
                gelus = []
                for s, w in chunks:
                    acc_full = ps_dn.tile([128, 512], F32, tag="dn")
                    acc = acc_full[:, :w]
                    for c in range(CTILES):
                        nc.tensor.matmul(
                            acc[:],
                            wdn_sb[:, c, :],
                            xsb[:, c, s : s + w],
                            start=(c == 0),
                            stop=(c == CTILES - 1),
                        )
                    g = nc.scalar.activation(
                        dng[:, s : s + w],
                        acc[:],
                        mybir.ActivationFunctionType.Gelu,
                        bias=bdn_sb[:],
                        scale=1.0,
                    )
                    gelus.append(g)
                return dng, gelus

            def attn_scores(b, dng):
                """Logits, exp, transposes, exp partial sums. Emitted BEFORE
                down(b+1) so the exps fill the ACT while down(b+1) waits on
                its x load; the PE cost here is small (~3us)."""
                exps = ex.tile([128, TTILES, P], BF, tag="exps", name="exps")
                toktr = tt.tile([128, TTILES, 128], BF, tag="toktr", name="toktr")
                poT = ps_po.tile([128, P + 8], F32, tag="po", name="poT")
                accA = red.tile([128, P], F32, tag="accA", name="accA")
                accB = red.tile([128, P], F32, tag="accB", name="accB")
                exp_insts = []
                st = {
                    "exps": exps, "toktr": toktr, "poT": poT,
                    "accA": accA, "accB": accB, "exp_insts": exp_insts,
                }
                for j in range(TTILES // 2):
                    t0, t1 = 2 * j, 2 * j + 1
                    lg = ps_lg.tile([128, 2, P], F32, tag="lg")
                    for k, t in ((0, t0), (1, t1)):
                        tok = dng[:, P + t * 128 : P + (t + 1) * 128]
                        nc.tensor.matmul(
                            lg[:, k, :], tok, dng[:, 0:P], start=True, stop=True
                        )
                    e = nc.scalar.activation(
                        exps[:, t0 : t0 + 2, :],
                        lg[:],
                        mybir.ActivationFunctionType.Exp,
                        scale=SCALE,
                    )
                    exp_insts.append(e)
                    # pair of PE transposes into the dedicated bf16 bank,
                    # one DVE copy for both
                    trp = ps_tr.tile([128, 2, 128], BF, tag="tr")
                    for k, t in ((0, t0), (1, t1)):
                        tok = dng[:, P + t * 128 : P + (t + 1) * 128]
                        nc.tensor.transpose(trp[:, k, :], tok, id_sb[:])
                    nc.vector.tensor_copy(toktr[:, t0 : t0 + 2, :], trp[:])
                    # exp partial sums: accA chain on DVE, accB on GpSimd
                    if j == 0:
                        nc.vector.tensor_add(accA[:], exps[:, 0, :], exps[:, 1, :])
                    elif j == 1:
                        nc.gpsimd.tensor_add(accB[:], exps[:, 2, :], exps[:, 3, :])
                    else:
                        nc.vector.tensor_add(accA[:], accA[:], exps[:, 2 * j, :])
                        nc.gpsimd.tensor_add(accB[:], accB[:], exps[:, 2 * j + 1, :])
                return st

            def tail(b, dng, poT, accA, accB):
                """Attention tail: denominator, reciprocal, prompt up-proj
                with normalization folded into the epilogue scale."""
                # accA += accB on GpSimd (SBUF-only) halves the den matmuls
                nc.gpsimd.tensor_add(accA[:], accA[:], accB[:])
                nc.tensor.matmul(
                    poT[:, P : P + 1], accA[:, 0:128], onesf_sb[:],
                    start=True, stop=True,
                )
                nc.tensor.matmul(
                    poT[0:72, P + 1 : P + 2], accA[:, 128:200], onesf_sb[:],
                    start=True, stop=True,
                )
                rec0 = sm.tile([128, 1], F32, tag="rec0")
                nc.vector.reciprocal(rec0[:], poT[:, P : P + 1])
                rec1 = sm.tile([72, 1], F32, tag="rec1")
                nc.vector.reciprocal(rec1[:], poT[0:72, P + 1 : P + 2])
                # unnormalized prompt_out -> dng's prompt region (DVE)
                nc.vector.tensor_copy(dng[:, 0:P], poT[:, 0:P])

                osbp = ob.tile([128, 4, 2, 512], BF, tag="osb")
                for h in range(2):
                    upa = ps_up.tile([128, 512], F32, tag="up")
                    nc.tensor.matmul(
                        upa[:],
                        dng[:, 0:128],
                        wup_sb[:, h * 512 : (h + 1) * 512],
                        start=True, stop=True,
                    )
                    nc.scalar.mul(osbp[:, 0, h, :], upa[:], rec0[:])
                    upb = ps_up.tile([128, 512], F32, tag="up")
                    nc.tensor.matmul(
                        upb[0:72, :],
                        dng[:, 128:200],
                        wup_sb[:, h * 512 : (h + 1) * 512],
                        start=True, stop=True,
                    )
                    nc.vector.tensor_scalar_mul(
                        osbp[0:72, 1, h, :], upb[0:72, :], rec1[:]
                    )
                nc.gpsimd.dma_start(
                    out[b, 0:128, :], osbp[:, 0].rearrange("p a b -> p (a b)")
                )
                nc.gpsimd.dma_start(
                    out[b, 128:200, :], osbp[0:72, 1].rearrange("p a b -> p (a b)")
                )

            def attn_finish(b, dng, st, late_up=False):
                """po accumulation block, tail, then the token up-projection
                stream. late_up (last batch): borrow the idle down/logits/
                transpose banks and store at 2-tile granularity."""
                exps, toktr, poT = st["exps"], st["toktr"], st["poT"]

                def po_and_tail():
                    for t in range(TTILES):
                        nc.tensor.matmul(
                            poT[:, 0:P],
                            toktr[:, t, :],
                            exps[:, t, :],
                            start=(t == 0),
                            stop=(t == TTILES - 1),
                        )
                    tail(b, dng, poT, st["accA"], st["accB"])

                if late_up:
                    # drain mode: the long po->den->prompt-up chain first so
                    # it overlaps the final up/cast/store stream
                    po_and_tail()

                borrow = [
                    (ps_up, "up"), (ps_up, "up"), (ps_dn, "dn"), (ps_dn, "dn"),
                    (ps_lg, "lg"), (ps_lg, "lg"), (ps_tr, "tr"),
                ]
                bctr = [0]
                osb = None
                for t in range(TTILES):
                    tok = dng[:, P + t * 128 : P + (t + 1) * 128]
                    q = t % 4
                    if q == 0:
                        osb = ob.tile([128, 4, 2, 512], BF, tag="osb")
                    # single-bank up tiles; the PSUM->SBUF casts run on ACT
                    # and DVE in parallel so each bank frees fast
                    ha = t % 2
                    for h in range(2):
                        if late_up:
                            pool, tg = borrow[bctr[0] % len(borrow)]
                            bctr[0] += 1
                            up = pool.tile([128, 512], F32, tag=tg)
                        else:
                            up = ps_up.tile([128, 512], F32, tag="up")
                        nc.tensor.matmul(
                            up[:],
                            tok,
                            wup_sb[:, h * 512 : (h + 1) * 512],
                            start=True,
                            stop=True,
                        )
                        if h == ha:
                            nc.scalar.copy(osb[:, q, h, :], up[:])
                        else:
                            nc.vector.tensor_copy(osb[:, q, h, :], up[:])
                    if late_up and q in (1, 3):
                        g2 = t // 2
                        dstd = out[
                            b, P + 256 * g2 : P + 256 * (g2 + 1), :
                        ].rearrange("(a p) c -> p a c", p=128)
                        lo = 0 if q == 1 else 2
                        nc.gpsimd.dma_start(
                            dstd,
                            osb[:, lo : lo + 2].rearrange("p a b c -> p a (b c)"),
                        )
                    elif not late_up and q == 3:
                        g4 = t // 4
                        dstd = out[
                            b, P + 512 * g4 : P + 512 * (g4 + 1), :
                        ].rearrange("(a p) c -> p a c", p=128)
                        nc.gpsimd.dma_start(
                            dstd, osb[:].rearrange("p a b c -> p a (b c)")
                        )
                if not late_up:
                    po_and_tail()

            # ---- software pipeline ----
            # per iteration b: [load x(b+2)] [attn_scores(b): logits+exps]
            # [down(b+1)] [attn_finish(b): po, tail, up+stores]
            # ACT order G0 E0 G1 E1 ... : each E(b) needs only logits(b), so
            # it runs while down(b+1) waits on its x load.
            load_x(0, split=True)
            load_x(1)
            dngs = {}
            gelu_groups = {}
            exp_groups = {}
            # batch 0's leading chunks are finer so the PE starts as soon as
            # the first 128 columns of x(0) land (~2us instead of ~7us)
            CHUNKS0 = [(0, 128), (128, 128), (256, 256)] + DOWN_CHUNKS[1:]
            dngs[0], gelu_groups[0] = down(0, chunks=CHUNKS0)
            for b in range(NB):
                if b + 2 < NB:
                    load_x(b + 2)
                st = attn_scores(b, dngs[b])
                exp_groups[b] = st["exp_insts"]
                if b + 1 < NB:
                    dngs[b + 1], gelu_groups[b + 1] = down(b + 1)
                attn_finish(b, dngs[b], st, late_up=(b == NB - 1))

            # ACT table discipline: pin the per-engine order G0 E0 G1 E1 ...
            # (gelu/exp alternate once per batch; per-instruction edges so
            # the scheduler cannot interleave the groups)
            for b in range(NB):
                g_last = gelu_groups[b][-1].ins
                for e in exp_groups[b]:
                    add_dep_helper(
                        e.ins, g_last, sync=False,
                        reason="ACT order: exps(b) after gelus(b)",
                    )
                if b + 1 < NB:
                    e_last = exp_groups[b][-1].ins
                    for g in gelu_groups[b + 1]:
                        add_dep_helper(
                            g.ins, e_last, sync=False,
                            reason="ACT order: gelus(b+1) after exps(b)",
                        )

    nc.compile()
    return nc


_NC_CACHE = None


def _get_nc():
    global _NC_CACHE
    if _NC_CACHE is None:
        _NC_CACHE = build_nc()
    return _NC_CACHE


def make_in_maps(x, W_down, b_down, W_up, b_up, gate):
    x = np.asarray(x, np.float32)
    W_down = np.asarray(W_down, np.float32)
    b_down = np.asarray(b_down, np.float32)
    W_up = np.asarray(W_up, np.float32)
    b_up = np.asarray(b_up, np.float32)
    gate = float(np.asarray(gate, np.float32))

    bf = ml_dtypes.bfloat16
    xT = np.ascontiguousarray(x.transpose(0, 2, 1)).astype(bf)  # [B, C, N]
    # wdn[p, c, m] = W_down[c*128 + p, m]
    wdn = np.ascontiguousarray(
        W_down.reshape(CTILES, 128, 128).transpose(1, 0, 2)
    ).astype(bf)
    wup = (W_up * gate).astype(bf)  # [D, C]
    bdn = b_down.reshape(D, 1).copy()
    ident = np.eye(128, dtype=bf)
    onesf = np.ones((128, 1), dtype=np.float32)

    in_maps = []
    for i in range(NCORES):
        in_maps.append(
            {
                "xT": np.ascontiguousarray(xT[i * NB : (i + 1) * NB]),
                "wdn": wdn,
                "wup": wup,
                "bdn": bdn,
                "ident": ident,
                "onesf": onesf,
            }
        )
    return in_maps


def kernel(**inputs):
    nc = _get_nc()
    in_maps = make_in_maps(**inputs)
    res = run_bass_kernel_spmd(nc, in_maps, core_ids=list(range(NCORES)))
    out = np.concatenate([res.results[i]["out"] for i in range(NCORES)], axis=0)
    out = out.astype(np.float32)
    # b_up (and gate) folded in on the host: device computes comb @ (gate*W_up)
    bias = (
        np.asarray(inputs["b_up"], np.float32)
        * float(np.asarray(inputs["gate"], np.float32))
    ).reshape(1, 1, C)
    return out + bias
